# revision 13
# baseline (speedup 1.0000x reference)
"""Biquad lowpass (torchaudio-style) over [64, 320000] via 128-tap FIR on TRN2.

The IIR's poles have |z| = sqrt(a2) ~ 0.870, so the impulse response decays
below fp32 epsilon after ~128 taps: the filter is exactly (to fp32) a 128-tap
causal FIR. In a time-transposed layout (time-within-block on partitions,
block index on the free dim) the FIR is two block-Toeplitz matmuls per output
block:  y_j = T0^T x_j + T1^T x_{j-1}.

Sharding: batch (64 sequences) x 8 cores = 8 sequences/core.
Host pre-transposes each sequence [2500,128] -> [128,2500] and splits x into
fp16 hi/lo halves (same total bytes as fp32). The device runs 6 fp16 matmuls
per 500-column chunk (hi/lo x cross T0h/T1h/T0l/T1l, dropping the lo*lo term),
accumulating in fp32 PSUM -> ~1e-6 relative error.
"""

import math

import numpy as np

B = 64
T = 320000
NCORES = 8
SEQ_PER_CORE = B // NCORES  # 8
BLK = 128
J = T // BLK  # 2500 blocks/sequence
COLS = SEQ_PER_CORE * J  # 20000 columns/core
CHUNK = 500  # moving-operand columns per matmul (<=512 for one PSUM bank)
CHUNKS_PER_SEQ = J // CHUNK  # 5
SEQ_PER_TILE = 1  # sequences per SBUF tile (per x double-buffer slot)
IN_PIECE = 1250  # columns per input DMA (steady state)
IN_PIECE_FIRST = 500  # columns per input DMA for the first tile (fast ramp-in)
OUT_PIECE = 1250  # columns per output DMA
XBUFS = 4
YBUFS = 4
OUT_ENG = "scalar"  # which HWDGE ring carries output DMAs: "sync" or "scalar"
EVAC = "alt"  # PSUM->SBUF copy engine: "alt", "vector", "scalar"
USE_FP8_LO = True  # ship xlo as scaled fp8e4 (25% less input DMA, ~2e-5 rel err)
LO_SCALE = 2048.0  # 2^11: exact power-of-2; xlo8 = xlo*LO_SCALE, T*8 = T*/LO_SCALE


SAMPLE_RATE = 16000
CUTOFF = 7500.0
Q = 0.707


def _coeffs():
    w0 = 2.0 * math.pi * CUTOFF / SAMPLE_RATE
    alpha = math.sin(w0) / (2.0 * Q)
    cos_w0 = math.cos(w0)
    b0 = (1.0 - cos_w0) / 2.0
    b1 = 1.0 - cos_w0
    b2 = b0
    a0 = 1.0 + alpha
    a1 = -2.0 * cos_w0
    a2 = 1.0 - alpha
    return (b0 / a0, b1 / a0, b2 / a0, a1 / a0, a2 / a0)


def _impulse_response(n):
    b0, b1, b2, a1, a2 = _coeffs()
    h = np.zeros(n, np.float64)
    x1 = x2 = y1 = y2 = 0.0
    for i in range(n):
        xn = 1.0 if i == 0 else 0.0
        yn = b0 * xn + b1 * x1 + b2 * x2 - a1 * y1 - a2 * y2
        h[i] = yn
        x2, x1 = x1, xn
        y2, y1 = y1, yn
    return h


def _toeplitz():
    h = _impulse_response(BLK)
    s = np.arange(BLK)[:, None]
    t = np.arange(BLK)[None, :]
    k0 = t - s
    T0 = np.where((k0 >= 0) & (k0 < BLK), h[np.clip(k0, 0, BLK - 1)], 0.0)
    k1 = BLK + t - s
    T1 = np.where((k1 >= 1) & (k1 < BLK), h[np.clip(k1, 0, BLK - 1)], 0.0)
    T0 = T0.astype(np.float32)
    T1 = T1.astype(np.float32)
    out = np.zeros((BLK, 4 * BLK), np.float16)
    out[:, 0:BLK] = T0.astype(np.float16)
    out[:, BLK : 2 * BLK] = T1.astype(np.float16)
    out[:, 2 * BLK : 3 * BLK] = (T0 - out[:, 0:BLK].astype(np.float32)).astype(
        np.float16
    )
    out[:, 3 * BLK : 4 * BLK] = (T1 - out[:, BLK : 2 * BLK].astype(np.float32)).astype(
        np.float16
    )
    return out


_NC_CACHE = {}


def _build_program():
    if "nc" in _NC_CACHE:
        return _NC_CACHE["nc"]
    import concourse.mybir as mybir
    from concourse import bacc
    from concourse.tile import TileContext

    f16 = mybir.dt.float16
    f32 = mybir.dt.float32
    f8 = mybir.dt.float8e4
    lo_dt = f8 if USE_FP8_LO else f16

    nc = bacc.Bacc("TRN2", target_bir_lowering=False)
    xhi_d = nc.dram_tensor("xhi", [BLK, COLS], f16, kind="ExternalInput")
    xlo_d = nc.dram_tensor("xlo", [BLK, COLS], lo_dt, kind="ExternalInput")
    tk_d = nc.dram_tensor("tk", [BLK, 4 * BLK], f16, kind="ExternalInput")
    tk8_d = nc.dram_tensor("tk8", [BLK, 2 * BLK], f8, kind="ExternalInput")
    y_d = nc.dram_tensor("y", [BLK, COLS], f32, kind="ExternalOutput")

    with TileContext(nc) as tc:
        with (
            tc.tile_pool(name="const", bufs=1) as cpool,
            tc.tile_pool(name="xs", bufs=XBUFS) as xpool,
            tc.tile_pool(name="ys", bufs=YBUFS) as ypool,
            tc.tile_pool(name="ps", bufs=8, space="PSUM") as ppool,
        ):
            tk = cpool.tile([BLK, 4 * BLK], f16)
            nc.sync.dma_start(tk[:], tk_d[:])
            T0h = tk[:, 0:BLK]
            T1h = tk[:, BLK : 2 * BLK]
            T0l = tk[:, 2 * BLK : 3 * BLK]
            T1l = tk[:, 3 * BLK : 4 * BLK]
            tk8 = cpool.tile([BLK, 2 * BLK], f8)
            nc.sync.dma_start(tk8[:], tk8_d[:])
            T0h_lo = tk8[:, 0:BLK] if USE_FP8_LO else T0h
            T1h_lo = tk8[:, BLK : 2 * BLK] if USE_FP8_LO else T1h

            evac = 0
            assert SEQ_PER_CORE % SEQ_PER_TILE == 0
            tcols = SEQ_PER_TILE * J
            for sg in range(SEQ_PER_CORE // SEQ_PER_TILE):
                t_lo = sg * tcols  # column base of this seq-group in DRAM
                xh = xpool.tile([BLK, tcols], f16, tag="xh")
                xl = xpool.tile([BLK, tcols], lo_dt, tag="xl")
                piece = IN_PIECE_FIRST if sg == 0 else IN_PIECE
                for p0 in range(0, tcols, piece):
                    nc.sync.dma_start(
                        xh[:, p0 : p0 + piece], xhi_d[:, t_lo + p0 : t_lo + p0 + piece]
                    )
                    nc.sync.dma_start(
                        xl[:, p0 : p0 + piece], xlo_d[:, t_lo + p0 : t_lo + p0 + piece]
                    )
                yt = ypool.tile([BLK, tcols], f32, tag="y")
                flushed = 0
                for sq in range(SEQ_PER_TILE):
                    for c in range(CHUNKS_PER_SEQ):
                        c0 = sq * J + c * CHUNK  # chunk start within the tile
                        ps = ppool.tile([BLK, CHUNK], f32)
                        mm = nc.tensor.matmul
                        mm(ps[:], T0h, xh[:, c0 : c0 + CHUNK], start=True, stop=False)
                        mm(ps[:], T0h_lo, xl[:, c0 : c0 + CHUNK], start=False, stop=False)
                        mm(ps[:], T0l, xh[:, c0 : c0 + CHUNK], start=False, stop=False)
                        if c == 0:
                            # sequence start: x_{j-1} of the first block is 0
                            po = ps[:, 1:CHUNK]
                            w0, w1 = c0, c0 + CHUNK - 1
                        else:
                            po = ps[:]
                            w0, w1 = c0 - 1, c0 + CHUNK - 1
                        mm(po, T1h, xh[:, w0:w1], start=False, stop=False)
                        mm(po, T1h_lo, xl[:, w0:w1], start=False, stop=False)
                        mm(po, T1l, xh[:, w0:w1], start=False, stop=True)
                        use_vec = EVAC == "vector" or (EVAC == "alt" and evac % 2 == 0)
                        if use_vec:
                            nc.vector.tensor_copy(yt[:, c0 : c0 + CHUNK], ps[:])
                        else:
                            nc.scalar.copy(yt[:, c0 : c0 + CHUNK], ps[:])
                        evac += 1
                        # flush finished output columns once a piece is full
                        done = sq * J + (c + 1) * CHUNK
                        while flushed + OUT_PIECE <= done or (
                            done == tcols and flushed < done
                        ):
                            o1 = min(flushed + OUT_PIECE, done)
                            out_eng = nc.sync if OUT_ENG == "sync" else nc.scalar
                            out_eng.dma_start(
                                y_d[:, t_lo + flushed : t_lo + o1], yt[:, flushed:o1]
                            )
                            flushed = o1
    nc.compile()
    _NC_CACHE["nc"] = nc
    return nc


def kernel(x: np.ndarray) -> np.ndarray:
    from concourse.bass_utils import run_bass_kernel_spmd

    assert x.shape == (B, T) and x.dtype == np.float32
    nc = _build_program()

    # host: per-sequence transpose [2500,128] -> [128,2500], hi/lo fp16 split
    import ml_dtypes

    xt = np.ascontiguousarray(
        x.reshape(B, J, BLK).transpose(0, 2, 1)
    )  # [B, 128, 2500]
    xhi = xt.astype(np.float16)
    resid = xt - xhi.astype(np.float32)
    if USE_FP8_LO:
        xlo = (resid * LO_SCALE).astype(ml_dtypes.float8_e4m3)
    else:
        xlo = resid.astype(np.float16)
    tk = _toeplitz()
    tk8 = np.zeros((BLK, 2 * BLK), ml_dtypes.float8_e4m3)
    tk8[:, 0:BLK] = (tk[:, 0:BLK].astype(np.float32) / LO_SCALE).astype(
        ml_dtypes.float8_e4m3
    )
    tk8[:, BLK : 2 * BLK] = (
        tk[:, BLK : 2 * BLK].astype(np.float32) / LO_SCALE
    ).astype(ml_dtypes.float8_e4m3)

    in_maps = []
    for c in range(NCORES):
        sl = slice(c * SEQ_PER_CORE, (c + 1) * SEQ_PER_CORE)
        in_maps.append(
            {
                "xhi": np.ascontiguousarray(
                    xhi[sl].transpose(1, 0, 2).reshape(BLK, COLS)
                ),
                "xlo": np.ascontiguousarray(
                    xlo[sl].transpose(1, 0, 2).reshape(BLK, COLS)
                ),
                "tk": tk,
                "tk8": tk8,
            }
        )

    res = run_bass_kernel_spmd(nc, in_maps, core_ids=list(range(NCORES)))

    y = np.empty((B, T), np.float32)
    for c in range(NCORES):
        yc = res.results[c]["y"]  # [128, 20000]
        y[c * SEQ_PER_CORE : (c + 1) * SEQ_PER_CORE] = (
            yc.reshape(BLK, SEQ_PER_CORE, J).transpose(1, 2, 0).reshape(SEQ_PER_CORE, T)
        )
    return y
